# revision 1
# baseline (speedup 1.0000x reference)
"""Bass/Trainium2 kernel for nn_GCA (graph attention message passing layer).

Strategy (8 NeuronCores, SPMD):
  - Nodes row-sharded: core m owns original nodes [5000m, 5000(m+1)), padded
    to 5120 slots per core (40 chunks x 128).
  - Phase A (per core): LN1 + q/k/v projections for the local 5120 rows.
    q stays in SBUF; k,v written interleaved to a DRAM bounce [5120, 256].
  - AllGather of the kv bounce -> kv_full [40960, 256] (halo exchange).
  - Edges partitioned by destination core, grouped into 128-dst-node chunks,
    padded to a uniform per-chunk capacity so the SPMD program is identical
    on every core. Per chunk: two dma_gathers (int16 indices; table split at
    row 8192 so indices fit int16) fetch k||v rows of the edge sources; per
    128-edge tile a one-hot matrix (dst_rel vs iota) built on DVE routes
    per-edge exp(q.k/4) weighted v into a per-chunk PSUM accumulator via
    TensorE matmuls (scatter-add as matmul).  The softmax is global over all
    edges (faithful to the reference), so aggregation is un-normalized and
    per-head sums Z are accumulated alongside, AllReduced (add) across cores,
    and divided out afterwards.
  - Phase C (per chunk): aggregated @ Wo + residual, LN2, FFN, residual,
    write the 5000 real rows of the output.
"""

import math
import os

import numpy as np

import concourse.bass as bass
import concourse.bacc as bacc
import concourse.tile as tile
from concourse import mybir
from concourse import bass_utils

F32 = mybir.dt.float32
BF16 = mybir.dt.bfloat16
I16 = mybir.dt.int16
AF = mybir.ActivationFunctionType
OP = mybir.AluOpType

M = 8            # cores
N = 40000        # nodes
C = 128          # channels
H = 8            # heads
D = 16           # head dim
E = 640000       # edges
FF = 512         # ffn dim
NPC = N // M     # 5000 nodes per core
NCH = 40         # chunks per core
NPAD = NCH * 128  # 5120 padded nodes per core
SPLIT = 8192     # table-B base row (int16 index headroom: 40960-8192 < 32768)
EPS = 1e-5
KLIMIT = int(os.environ.get("KLIMIT", str(NCH)))
KABL = os.environ.get("KABL", "")
GB = 4  # edge tiles batched per DVE op


def _preprocess(x, edge_index):
    """Host-side sharding: returns per-core arrays + capacities."""
    src = np.asarray(edge_index[0], dtype=np.int64)
    dst = np.asarray(edge_index[1], dtype=np.int64)

    core = dst // NPC
    dst_loc = dst - core * NPC
    ch = dst_loc >> 7                    # 128-node chunk within core
    dst_rel = dst_loc & 127
    src_kv = (src // NPC) * NPAD + (src % NPC)
    hi = (src_kv >= 32768).astype(np.int64)

    grp = (core * NCH + ch) * 2 + hi     # [E]
    order = np.argsort(grp, kind="stable")
    grp_s = grp[order]
    uniq, first = np.unique(grp_s, return_index=True)
    pos = np.arange(E) - first[np.searchsorted(uniq, grp_s)]

    m_s = core[order]
    c_s = ch[order]
    hi_s = hi[order]
    dr_s = dst_rel[order]
    kv_s = src_kv[order]

    n_grp = np.zeros(M * NCH * 2, dtype=np.int64)
    np.add.at(n_grp, grp_s, 1)
    n_lo = n_grp[0::2].max()
    n_hi = n_grp[1::2].max()
    cap_a = max(128, math.ceil(n_lo / 128) * 128)
    cap_b = max(128, math.ceil(n_hi / 128) * 128)
    ta, tb = cap_a // 128, cap_b // 128
    tt = ta + tb

    # slot within chunk: A edges at tiles [0, ta), B edges at [ta, tt)
    t_s = np.where(hi_s == 0, pos >> 7, ta + (pos >> 7))
    p_s = pos & 127

    dstc = np.zeros((M, NCH, 128, tt), dtype=np.float32)
    mask = np.zeros((M, NCH, 128, tt), dtype=np.float32)
    dstc[m_s, c_s, p_s, t_s] = dr_s.astype(np.float32)
    mask[m_s, c_s, p_s, t_s] = 1.0

    idx_a = np.zeros((M, NCH, cap_a), dtype=np.int16)
    idx_b = np.zeros((M, NCH, cap_b), dtype=np.int16)
    lo_m = hi_s == 0
    idx_a[m_s[lo_m], c_s[lo_m], pos[lo_m]] = kv_s[lo_m].astype(np.int16)
    hi_m = ~lo_m
    idx_b[m_s[hi_m], c_s[hi_m], pos[hi_m]] = (kv_s[hi_m] - SPLIT).astype(np.int16)

    def wrap(a, cap):  # [..., cap] -> [..., 128, cap//16] (16-wrap, x8 replicate)
        w = a.reshape(M, NCH, cap // 16, 16).swapaxes(-1, -2)  # [M,NCH,16,cap/16]
        return np.ascontiguousarray(np.tile(w, (1, 1, 8, 1)))

    idx_a = wrap(idx_a, cap_a)
    idx_b = wrap(idx_b, cap_b)

    x_loc = np.zeros((M, NPAD, C), dtype=np.float32)
    x_loc[:, :NPC] = np.asarray(x, dtype=np.float32).reshape(M, NPC, C)

    return x_loc, idx_a, idx_b, dstc, mask, cap_a, cap_b


def _build(cap_a, cap_b):
    ta, tb = cap_a // 128, cap_b // 128
    tt = ta + tb
    nc = bacc.Bacc("TRN2", target_bir_lowering=False, debug=False,
                   num_devices=M)

    din = {}
    for name, shape in [
        ("x_loc", [NPAD, C]), ("wq", [C, C]), ("wk", [C, C]), ("wv", [C, C]),
        ("wo", [C, C]), ("w1", [C, FF]), ("w2s", [4, C, C]),
        ("dstc", [NCH, 128, tt]), ("mask", [NCH, 128, tt]),
        ("g1", [1, C]), ("b1ln", [1, C]), ("g2", [1, C]), ("b2ln", [1, C]),
        ("bq", [1, C]), ("bk", [1, C]), ("bv", [1, C]), ("bo", [1, C]),
        ("b1f", [1, FF]), ("b2f", [1, C]),
        ("ident", [128, 128]), ("iota_r", [1, 128]), ("expand", [32, 128]),
    ]:
        din[name] = nc.dram_tensor(name, shape, F32, kind="ExternalInput")
    din["idx_a"] = nc.dram_tensor("idx_a", [NCH, 128, cap_a // 16], I16,
                                  kind="ExternalInput")
    din["idx_b"] = nc.dram_tensor("idx_b", [NCH, 128, cap_b // 16], I16,
                                  kind="ExternalInput")
    out_d = nc.dram_tensor("out", [NPAD, C], F32, kind="ExternalOutput")

    kv_bounce = nc.dram_tensor("kv_bounce", [NPAD, 2 * C], F32)
    kv_full = nc.dram_tensor("kv_full", [M * NPAD, 2 * C], F32,
                             addr_space="Shared")
    z_bounce = nc.dram_tensor("z_bounce", [128, GB * H], F32)
    z_red = nc.dram_tensor("z_red", [128, GB * H], F32, addr_space="Shared")

    with tile.TileContext(nc) as tc:
        with (
            tc.tile_pool(name="consts", bufs=1) as cp,
            tc.tile_pool(name="persist", bufs=1) as pp,
            tc.tile_pool(name="work", bufs=3) as wp,
            tc.tile_pool(name="kvgp", bufs=2) as kvp,
            tc.tile_pool(name="edge4", bufs=4) as e4,
            tc.tile_pool(name="ps_t", bufs=3, space="PSUM") as ps_t,
            tc.tile_pool(name="ps_qg", bufs=2, space="PSUM") as ps_qg,
            tc.tile_pool(name="ps_agg", bufs=2, space="PSUM") as ps_agg,
            tc.tile_pool(name="ps_big", bufs=1, space="PSUM") as ps_big,
        ):
            # ---- constants ----
            def bload(name, cols, rows=128):
                t = cp.tile([rows, cols], F32, tag=name)
                src = din[name].ap()
                bc = bass.AP(tensor=src.tensor, offset=0,
                             ap=[[0, rows]] + list(src.ap[1:]))
                nc.sync.dma_start(out=t[:], in_=bc)
                return t

            g1_b = bload("g1", C)
            b1ln_b = bload("b1ln", C)
            g2_b = bload("g2", C)
            b2ln_b = bload("b2ln", C)
            bq_b = bload("bq", C)
            bk_b = bload("bk", C)
            bv_b = bload("bv", C)
            bo_b = bload("bo", C)
            b1f_b = bload("b1f", FF)
            b2f_b = bload("b2f", C)
            iota_b = bload("iota_r", 128)

            wq_s = cp.tile([C, C], F32, tag="wq")
            nc.sync.dma_start(out=wq_s[:], in_=din["wq"][:])
            wk_s = cp.tile([C, C], F32, tag="wk")
            nc.sync.dma_start(out=wk_s[:], in_=din["wk"][:])
            wv_s = cp.tile([C, C], F32, tag="wv")
            nc.sync.dma_start(out=wv_s[:], in_=din["wv"][:])
            wo_s = cp.tile([C, C], F32, tag="wo")
            nc.sync.dma_start(out=wo_s[:], in_=din["wo"][:])
            w1_s = cp.tile([C, FF], F32, tag="w1")
            nc.sync.dma_start(out=w1_s[:], in_=din["w1"][:])
            w2_s = cp.tile([C, 4 * C], F32, tag="w2")
            for j in range(4):
                nc.sync.dma_start(out=w2_s[:, j * C:(j + 1) * C],
                                  in_=din["w2s"][j])
            ident_s = cp.tile([128, 128], F32, tag="ident")
            nc.sync.dma_start(out=ident_s[:], in_=din["ident"][:])
            expand_s = cp.tile([32, 128], F32, tag="expand")
            nc.sync.dma_start(out=expand_s[:], in_=din["expand"][:])

            eps_t = cp.tile([128, 1], F32, tag="eps")
            nc.vector.memset(eps_t[:], EPS)
            ones_t = cp.tile([128, 1], F32, tag="ones")
            nc.vector.memset(ones_t[:], 1.0)
            z_acc = pp.tile([128, GB * H], F32, tag="z_acc")
            nc.vector.memset(z_acc[:], 0.0)

            x_tiles = [pp.tile([128, C], F32, name=f"x{i}", tag=f"x{i}") for i in range(NCH)]
            q_tiles = [pp.tile([128, C], F32, name=f"q{i}", tag=f"q{i}") for i in range(NCH)]
            a_tiles = [pp.tile([128, C], F32, name=f"a{i}", tag=f"a{i}") for i in range(NCH)]

            def layernorm(x_t, out_t, g_b, b_b):
                st = wp.tile([128, 6], F32, tag="ln_st")
                mv = wp.tile([128, 2], F32, tag="ln_mv")
                nc.vector.bn_stats(out=st[:], in_=x_t[:])
                nc.vector.bn_aggr(out=mv[:], in_=st[:])
                std = wp.tile([128, 1], F32, tag="ln_std")
                nc.scalar.activation(out=std[:], in_=mv[:, 1:2], func=AF.Sqrt,
                                     bias=eps_t[:], scale=1.0)
                nc.vector.reciprocal(out=std[:], in_=std[:])
                nc.vector.tensor_scalar(out=out_t[:], in0=x_t[:],
                                        scalar1=mv[:, 0:1], scalar2=std[:],
                                        op0=OP.subtract, op1=OP.mult)
                nc.vector.tensor_mul(out=out_t[:], in0=out_t[:], in1=g_b[:])
                nc.vector.tensor_add(out=out_t[:], in0=out_t[:], in1=b_b[:])

            # ================= Phase A =================
            for i in range(NCH):
                x_t = x_tiles[i]
                nc.sync.dma_start(out=x_t[:],
                                  in_=din["x_loc"][i * 128:(i + 1) * 128, :])
                xn = wp.tile([128, C], F32, tag="xn")
                layernorm(x_t, xn, g1_b, b1ln_b)
                xnT_p = ps_t.tile([128, C], F32, tag="pt")
                nc.tensor.transpose(out=xnT_p[:], in_=xn[:], identity=ident_s[:])
                xnT = wp.tile([128, C], F32, tag="xnT")
                nc.vector.tensor_copy(out=xnT[:], in_=xnT_p[:])

                q_p = ps_qg.tile([128, C], F32, tag="pqg")
                nc.tensor.matmul(out=q_p[:], lhsT=xnT[:], rhs=wq_s[:],
                                 start=True, stop=True)
                nc.vector.tensor_add(out=q_tiles[i][:], in0=q_p[:], in1=bq_b[:])

                k_p = ps_agg.tile([128, C], F32, tag="pagg")
                nc.tensor.matmul(out=k_p[:], lhsT=xnT[:], rhs=wk_s[:],
                                 start=True, stop=True)
                kv_t = wp.tile([128, 2 * C], F32, tag="kv_t")
                nc.vector.tensor_add(out=kv_t[:, 0:C], in0=k_p[:], in1=bk_b[:])

                v_p = ps_big.tile([128, C], F32, tag="pbig")
                nc.tensor.matmul(out=v_p[:], lhsT=xnT[:], rhs=wv_s[:],
                                 start=True, stop=True)
                nc.vector.tensor_add(out=kv_t[:, C:2 * C], in0=v_p[:], in1=bv_b[:])

                nc.sync.dma_start(
                    out=kv_bounce[i * 128:(i + 1) * 128, :], in_=kv_t[:])

            # ================= AllGather kv =================
            if KABL != "nocoll":
                nc.gpsimd.collective_compute(
                    "AllGather", OP.bypass,
                    replica_groups=[list(range(M))],
                    ins=[kv_bounce[:].opt()],
                    outs=[kv_full[:].opt()],
                )

            # ================= Phase B: edges =================
            for c in range(KLIMIT):
                idxa_t = wp.tile([128, cap_a // 16], I16, tag="idxa")
                nc.sync.dma_start(out=idxa_t[:], in_=din["idx_a"][c])
                idxb_t = wp.tile([128, cap_b // 16], I16, tag="idxb")
                nc.sync.dma_start(out=idxb_t[:], in_=din["idx_b"][c])
                dstc_t = wp.tile([128, tt], F32, tag="dstc")
                nc.sync.dma_start(out=dstc_t[:], in_=din["dstc"][c])
                mask_t = wp.tile([128, tt], F32, tag="maskt")
                nc.sync.dma_start(out=mask_t[:], in_=din["mask"][c])

                kvg = kvp.tile([128, tt * 2 * C], F32, tag="kvg")
                kvg3 = kvg[:].rearrange("p (t c) -> p t c", t=tt)
                if KABL != "nogather":
                    nc.gpsimd.dma_gather(
                    out_ap=kvg3[:, 0:ta, :], in_ap=kv_full[:],
                    idxs_ap=idxa_t[:], num_idxs=cap_a, num_idxs_reg=cap_a,
                        elem_size=2 * C, single_packet=cap_a <= 1024)
                    nc.gpsimd.dma_gather(
                        out_ap=kvg3[:, ta:tt, :], in_ap=kv_full[SPLIT:, :],
                        idxs_ap=idxb_t[:], num_idxs=cap_b, num_idxs_reg=cap_b,
                        elem_size=2 * C, single_packet=cap_b <= 1024)

                aggT_p = ps_agg.tile([128, 128], F32, tag="pagg")
                q_c = q_tiles[c]
                inner_tt = 0 if KABL == "noinner" else tt
                groups = []
                t0 = 0
                while t0 < inner_tt:
                    groups.append((t0, min(GB, inner_tt - t0)))
                    t0 += GB
                for (t0, bs) in groups:
                    w = bs * 128
                    # one-hot for bs tiles in one DVE op: [128, bs, 128]
                    oh4 = e4.tile([128, GB * 128], F32, tag="oh4")
                    dap = dstc_t[:, t0:t0 + bs]
                    in0 = bass.AP(tensor=dap.tensor, offset=dap.offset,
                                  ap=list(dap.ap) + [[0, 128]])
                    iap = iota_b[:]
                    in1 = bass.AP(tensor=iap.tensor, offset=iap.offset,
                                  ap=[list(iap.ap[0]), [0, bs], list(iap.ap[1])])
                    oh4v = oh4[:, 0:w]
                    nc.vector.tensor_tensor(
                        out=oh4v.rearrange("p (b i) -> p b i", b=bs),
                        in0=in0, in1=in1, op=OP.is_equal)

                    # bs PE transposes into one PSUM bank, one copy out
                    ohT_p = ps_t.tile([128, GB * 128], F32, tag="pt")
                    for j in range(bs):
                        nc.tensor.transpose(
                            out=ohT_p[:, j * 128:(j + 1) * 128],
                            in_=oh4[:, j * 128:(j + 1) * 128],
                            identity=ident_s[:])
                    ohT4 = e4.tile([128, GB * 128], F32, tag="ohT4")
                    nc.vector.tensor_copy(out=ohT4[:, 0:w], in_=ohT_p[:, 0:w])

                    # bs q-expansion matmuls into one PSUM bank
                    qg_p = ps_qg.tile([128, GB * 128], F32, tag="pqg")
                    for j in range(bs):
                        nc.tensor.matmul(out=qg_p[:, j * 128:(j + 1) * 128],
                                         lhsT=ohT4[:, j * 128:(j + 1) * 128],
                                         rhs=q_c[:], start=True, stop=True)

                    prod4 = e4.tile([128, GB * 128], F32, tag="prod4")
                    s84 = e4.tile([128, GB * H], F32, tag="s84")
                    nc.vector.tensor_mul(
                        out=prod4[:, 0:w], in0=qg_p[:, 0:w],
                        in1=kvg3[:, t0:t0 + bs, 0:C])
                    nc.vector.reduce_sum(
                        out=s84[:, 0:bs * H].rearrange("p (b h) -> p b h", b=bs),
                        in_=prod4[:, 0:w].rearrange("p (b h d) -> p b h d",
                                                    b=bs, h=H),
                        axis=mybir.AxisListType.X)
                    w84 = e4.tile([128, GB * H], F32, tag="w84")
                    nc.scalar.activation(out=w84[:, 0:bs * H],
                                         in_=s84[:, 0:bs * H], func=AF.Exp,
                                         scale=1.0 / math.sqrt(D))
                    # mask: per (p, tile) broadcast over heads
                    map_ = mask_t[:, t0:t0 + bs]
                    mbc = bass.AP(tensor=map_.tensor, offset=map_.offset,
                                  ap=list(map_.ap) + [[0, H]])
                    nc.vector.tensor_tensor(
                        out=w84[:, 0:bs * H].rearrange("p (b h) -> p b h", b=bs),
                        in0=w84[:, 0:bs * H].rearrange("p (b h) -> p b h", b=bs),
                        in1=mbc, op=OP.mult)
                    if KABL != "noz":
                        nc.vector.tensor_add(out=z_acc[:, 0:bs * H],
                                             in0=z_acc[:, 0:bs * H],
                                             in1=w84[:, 0:bs * H])

                    wt4 = e4.tile([128, GB * 128], F32, tag="wt4")
                    w8ap = w84[:, 0:bs * H]
                    w8b = bass.AP(
                        tensor=w8ap.tensor, offset=w8ap.offset,
                        ap=[list(w8ap.ap[0]), [H, bs], [1, H], [0, D]])
                    nc.vector.tensor_tensor(
                        out=wt4[:, 0:w].rearrange("p (b h d) -> p b h d",
                                                  b=bs, h=H),
                        in0=kvg3[:, t0:t0 + bs, C:2 * C].rearrange(
                            "p b (h d) -> p b h d", h=H),
                        in1=w8b, op=OP.mult)

                    for j in range(bs):
                        t = t0 + j
                        nc.tensor.matmul(out=aggT_p[:],
                                         lhsT=wt4[:, j * 128:(j + 1) * 128],
                                         rhs=oh4[:, j * 128:(j + 1) * 128],
                                         start=(t == 0), stop=(t == inner_tt - 1))

                nc.vector.tensor_copy(out=a_tiles[c][:], in_=aggT_p[:])

            # ================= Z reduce =================
            nc.sync.dma_start(out=z_bounce[:], in_=z_acc[:])
            if KABL != "nocoll":
                nc.gpsimd.collective_compute(
                    "AllReduce", OP.add,
                    replica_groups=[list(range(M))],
                    ins=[z_bounce[:].opt()],
                    outs=[z_red[:].opt()],
                )
            z_red_s = wp.tile([128, GB * H], F32, tag="z_red_s")
            nc.sync.dma_start(out=z_red_s[:], in_=z_red[:])
            zcol_p = ps_t.tile([GB * H, 1], F32, tag="pt")
            nc.tensor.matmul(out=zcol_p[:], lhsT=z_red_s[:], rhs=ones_t[:],
                             start=True, stop=True)
            zs = wp.tile([GB * H, 1], F32, tag="zs")
            nc.vector.tensor_copy(out=zs[:], in_=zcol_p[:])
            zb_p = ps_t.tile([128, 1], F32, tag="pt")
            nc.tensor.matmul(out=zb_p[:], lhsT=expand_s[:], rhs=zs[:],
                             start=True, stop=True)
            zrecip = cp.tile([128, 1], F32, tag="zrecip")
            nc.vector.tensor_copy(out=zrecip[:], in_=zb_p[:])
            nc.vector.reciprocal(out=zrecip[:], in_=zrecip[:])

            # ================= Phase C =================
            for c in range(KLIMIT):
                sagg = wp.tile([128, 128], F32, tag="sagg")
                nc.vector.tensor_scalar_mul(out=sagg[:], in0=a_tiles[c][:],
                                            scalar1=zrecip[:])
                o_p = ps_qg.tile([128, C], F32, tag="pqg")
                nc.tensor.matmul(out=o_p[:], lhsT=sagg[:], rhs=wo_s[:],
                                 start=True, stop=True)
                x1 = wp.tile([128, C], F32, tag="x1")
                nc.vector.tensor_add(out=x1[:], in0=o_p[:], in1=bo_b[:])
                nc.vector.tensor_add(out=x1[:], in0=x1[:], in1=x_tiles[c][:])

                xn2 = wp.tile([128, C], F32, tag="xn2")
                layernorm(x1, xn2, g2_b, b2ln_b)
                xn2T_p = ps_t.tile([128, C], F32, tag="pt")
                nc.tensor.transpose(out=xn2T_p[:], in_=xn2[:],
                                    identity=ident_s[:])
                xn2T = wp.tile([128, C], F32, tag="xn2T")
                nc.vector.tensor_copy(out=xn2T[:], in_=xn2T_p[:])

                h_p = ps_big.tile([128, FF], F32, tag="pbig")
                nc.tensor.matmul(out=h_p[:], lhsT=xn2T[:], rhs=w1_s[:],
                                 start=True, stop=True)
                hb = wp.tile([128, FF], F32, tag="hb")
                nc.vector.tensor_add(out=hb[:], in0=h_p[:], in1=b1f_b[:])
                nc.vector.tensor_scalar_max(out=hb[:], in0=hb[:], scalar1=0.0)

                hT = wp.tile([128, FF], F32, tag="hT")
                for j in range(4):
                    hT_p = ps_t.tile([128, C], F32, tag="pt")
                    nc.tensor.transpose(out=hT_p[:],
                                        in_=hb[:, j * C:(j + 1) * C],
                                        identity=ident_s[:])
                    nc.vector.tensor_copy(out=hT[:, j * C:(j + 1) * C],
                                          in_=hT_p[:])
                f_p = ps_agg.tile([128, C], F32, tag="pagg")
                for j in range(4):
                    nc.tensor.matmul(out=f_p[:], lhsT=hT[:, j * C:(j + 1) * C],
                                     rhs=w2_s[:, j * C:(j + 1) * C],
                                     start=(j == 0), stop=(j == 3))
                fin = wp.tile([128, C], F32, tag="fin")
                nc.vector.tensor_add(out=fin[:], in0=f_p[:], in1=b2f_b[:])
                nc.vector.tensor_add(out=fin[:], in0=fin[:], in1=x1[:])
                nc.sync.dma_start(out=out_d[c * 128:(c + 1) * 128, :],
                                  in_=fin[:])

    nc.compile()
    return nc


_CACHE = {}


def _make_in_maps(inputs, x_loc, idx_a, idx_b, dstc, mask):
    f32 = lambda a: np.ascontiguousarray(np.asarray(a, dtype=np.float32))
    expand = np.zeros((32, 128), dtype=np.float32)
    for b in range(4):
        for h in range(H):
            expand[b * 8 + h, h * D:(h + 1) * D] = 1.0

    shared = dict(
        wq=f32(inputs["Wq"]), wk=f32(inputs["Wk"]), wv=f32(inputs["Wv"]),
        wo=f32(inputs["Wo"]), w1=f32(inputs["W1"]),
        w2s=f32(inputs["W2"]).reshape(4, C, C),
        g1=f32(inputs["ln1_g"]).reshape(1, C),
        b1ln=f32(inputs["ln1_b"]).reshape(1, C),
        g2=f32(inputs["ln2_g"]).reshape(1, C),
        b2ln=f32(inputs["ln2_b"]).reshape(1, C),
        bq=f32(inputs["bq"]).reshape(1, C), bk=f32(inputs["bk"]).reshape(1, C),
        bv=f32(inputs["bv"]).reshape(1, C), bo=f32(inputs["bo"]).reshape(1, C),
        b1f=f32(inputs["b1"]).reshape(1, FF),
        b2f=f32(inputs["b2"]).reshape(1, C),
        ident=np.eye(128, dtype=np.float32),
        iota_r=np.arange(128, dtype=np.float32).reshape(1, 128),
        expand=expand,
    )
    return [
        dict(shared, x_loc=x_loc[m], idx_a=idx_a[m], idx_b=idx_b[m],
             dstc=dstc[m], mask=mask[m])
        for m in range(M)
    ]


def kernel(**inputs):
    x = np.asarray(inputs["x"], dtype=np.float32)
    edge_index = np.asarray(inputs["edge_index"])

    x_loc, idx_a, idx_b, dstc, mask, cap_a, cap_b = _preprocess(x, edge_index)

    key = (cap_a, cap_b)
    if key not in _CACHE:
        _CACHE[key] = _build(cap_a, cap_b)
    nc = _CACHE[key]

    in_maps = _make_in_maps(inputs, x_loc, idx_a, idx_b, dstc, mask)
    res = bass_utils.run_bass_kernel_spmd(nc, in_maps, core_ids=list(range(M)))
    out = np.concatenate([res.results[m]["out"][:NPC] for m in range(M)], axis=0)
    return out.astype(np.float32)



# revision 4
# speedup vs baseline: 1.1653x; 1.1653x over previous
"""Bass/Trainium2 kernel for nn_GCA (graph attention message passing layer).

Strategy (8 NeuronCores, SPMD) — v2, bf16 edge path:
  - Nodes row-sharded: core m owns original nodes [5000m, 5000(m+1)), padded
    to 5120 slots per core (40 chunks x 128).
  - Phase A (per core): LN1 (fp32) + q/k/v projections in bf16 for the local
    5120 rows.  q stays in SBUF (bf16); k,v written interleaved to a DRAM
    bounce [5120, 256] bf16.
  - AllGather of the bf16 kv bounce -> kv_full [40960, 256] (halo exchange,
    half the bytes of fp32).
  - Edges partitioned by destination core, grouped into 128-dst-node chunks,
    padded to a uniform per-chunk capacity so the SPMD program is identical
    on every core.  Per chunk: two dma_gathers (int16 indices; table split at
    row 8192) fetch bf16 k||v rows (512 B/edge, at the DMA engines'
    no-read-modify-write descriptor size).  Per 128-edge tile a one-hot
    matrix (dst_rel vs iota, bf16) routes per-edge exp(q.k/4) weighted v
    into a per-chunk PSUM accumulator via bf16 TensorE matmuls.  PSUM
    evictions ride the Scalar (ACT) engine to keep DVE off the critical
    path.  The softmax is global over all edges (faithful to the reference):
    per-head weight sums Z accumulate in a tiny PSUM tile via an extra
    N=1 matmul per tile-group, are AllReduced (fp32, 128 B), and divided
    out afterwards.
  - Phase C (per chunk): aggregated @ Wo + residual, LN2, FFN (bf16
    matmuls), residual, write the 5000 real rows of the output (fp32).
"""

import math
import os

import numpy as np
import ml_dtypes

import concourse.bass as bass
import concourse.bacc as bacc
import concourse.tile as tile
from concourse import mybir
from concourse import bass_utils

F32 = mybir.dt.float32
BF16 = mybir.dt.bfloat16
I16 = mybir.dt.int16
AF = mybir.ActivationFunctionType
OP = mybir.AluOpType

M = 8            # cores
N = 40000        # nodes
C = 128          # channels
H = 8            # heads
D = 16           # head dim
E = 640000       # edges
FF = 512         # ffn dim
NPC = N // M     # 5000 nodes per core
NCH = 40         # chunks per core
NPAD = NCH * 128  # 5120 padded nodes per core
SPLIT = 8192     # table-B base row (int16 index headroom: 40960-8192 < 32768)
EPS = 1e-5
KLIMIT = int(os.environ.get("KLIMIT", str(NCH)))
KABL = os.environ.get("KABL", "")
GB = 4  # edge tiles batched per DVE op

BF = ml_dtypes.bfloat16


def _preprocess(x, edge_index):
    """Host-side sharding: returns per-core arrays + capacities."""
    src = np.asarray(edge_index[0], dtype=np.int64)
    dst = np.asarray(edge_index[1], dtype=np.int64)

    core = dst // NPC
    dst_loc = dst - core * NPC
    ch = dst_loc >> 7                    # 128-node chunk within core
    dst_rel = dst_loc & 127
    src_kv = (src // NPC) * NPAD + (src % NPC)
    hi = (src_kv >= 32768).astype(np.int64)

    grp = (core * NCH + ch) * 2 + hi     # [E]
    order = np.argsort(grp, kind="stable")
    grp_s = grp[order]
    uniq, first = np.unique(grp_s, return_index=True)
    pos = np.arange(E) - first[np.searchsorted(uniq, grp_s)]

    m_s = core[order]
    c_s = ch[order]
    hi_s = hi[order]
    dr_s = dst_rel[order]
    kv_s = src_kv[order]

    n_grp = np.zeros(M * NCH * 2, dtype=np.int64)
    np.add.at(n_grp, grp_s, 1)
    n_lo = n_grp[0::2].max()
    n_hi = n_grp[1::2].max()
    cap_a = max(128, math.ceil(n_lo / 128) * 128)
    cap_b = max(128, math.ceil(n_hi / 128) * 128)
    ta, tb = cap_a // 128, cap_b // 128
    tt = ta + tb

    # slot within chunk: A edges at tiles [0, ta), B edges at [ta, tt)
    t_s = np.where(hi_s == 0, pos >> 7, ta + (pos >> 7))
    p_s = pos & 127

    dstc = np.zeros((M, NCH, 128, tt), dtype=np.float32)
    mask = np.zeros((M, NCH, 128, tt), dtype=np.float32)
    dstc[m_s, c_s, p_s, t_s] = dr_s.astype(np.float32)
    mask[m_s, c_s, p_s, t_s] = 1.0

    idx_a = np.zeros((M, NCH, cap_a), dtype=np.int16)
    idx_b = np.zeros((M, NCH, cap_b), dtype=np.int16)
    lo_m = hi_s == 0
    idx_a[m_s[lo_m], c_s[lo_m], pos[lo_m]] = kv_s[lo_m].astype(np.int16)
    hi_m = ~lo_m
    idx_b[m_s[hi_m], c_s[hi_m], pos[hi_m]] = (kv_s[hi_m] - SPLIT).astype(np.int16)

    def wrap(a, cap):  # [..., cap] -> [..., 128, cap//16] (16-wrap, x8 replicate)
        w = a.reshape(M, NCH, cap // 16, 16).swapaxes(-1, -2)  # [M,NCH,16,cap/16]
        return np.ascontiguousarray(np.tile(w, (1, 1, 8, 1)))

    idx_a = wrap(idx_a, cap_a)
    idx_b = wrap(idx_b, cap_b)

    x_loc = np.zeros((M, NPAD, C), dtype=np.float32)
    x_loc[:, :NPC] = np.asarray(x, dtype=np.float32).reshape(M, NPC, C)

    return x_loc, idx_a, idx_b, dstc.astype(BF), mask.astype(BF), cap_a, cap_b


def _build(cap_a, cap_b):
    ta, tb = cap_a // 128, cap_b // 128
    tt = ta + tb
    nc = bacc.Bacc("TRN2", target_bir_lowering=False, debug=False,
                   num_devices=M)

    din = {}
    for name, shape in [
        ("x_loc", [NPAD, C]),
        ("g1", [1, C]), ("b1ln", [1, C]), ("g2", [1, C]), ("b2ln", [1, C]),
        ("bq", [1, C]), ("bk", [1, C]), ("bv", [1, C]), ("bo", [1, C]),
        ("b1f", [1, FF]), ("b2f", [1, C]),
        ("expand", [32, 128]),
    ]:
        din[name] = nc.dram_tensor(name, shape, F32, kind="ExternalInput")
    for name, shape in [
        ("wq", [C, C]), ("wk", [C, C]), ("wv", [C, C]), ("wo", [C, C]),
        ("w1", [C, FF]), ("w2s", [4, C, C]),
        ("dstc", [NCH, 128, tt]), ("mask", [NCH, 128, tt]),
        ("identb", [128, 128]), ("iota_r", [1, 128]),
    ]:
        din[name] = nc.dram_tensor(name, shape, BF16, kind="ExternalInput")
    din["idx_a"] = nc.dram_tensor("idx_a", [NCH, 128, cap_a // 16], I16,
                                  kind="ExternalInput")
    din["idx_b"] = nc.dram_tensor("idx_b", [NCH, 128, cap_b // 16], I16,
                                  kind="ExternalInput")
    out_d = nc.dram_tensor("out", [NPAD, C], F32, kind="ExternalOutput")

    kv_bounce = nc.dram_tensor("kv_bounce", [NPAD, 2 * C], BF16)
    kv_full = nc.dram_tensor("kv_full", [M * NPAD, 2 * C], BF16,
                             addr_space="Shared")
    z_bounce = nc.dram_tensor("z_bounce", [32, 1], F32)
    z_red = nc.dram_tensor("z_red", [32, 1], F32, addr_space="Shared")

    with tile.TileContext(nc) as tc:
        with (
            tc.tile_pool(name="consts", bufs=1) as cp,
            tc.tile_pool(name="persist", bufs=1) as pp,
            tc.tile_pool(name="work", bufs=3) as wp,
            tc.tile_pool(name="kvgp", bufs=2) as kvp,
            tc.tile_pool(name="edge4", bufs=4) as e4,
            tc.tile_pool(name="ps_t", bufs=2, space="PSUM") as ps_t,
            tc.tile_pool(name="ps_qg", bufs=2, space="PSUM") as ps_qg,
            tc.tile_pool(name="ps_agg", bufs=2, space="PSUM") as ps_agg,
            tc.tile_pool(name="ps_big", bufs=1, space="PSUM") as ps_big,
            tc.tile_pool(name="ps_z", bufs=1, space="PSUM") as ps_z,
        ):
            # ---- constants ----
            def bload(name, cols, rows=128, dt=F32):
                t = cp.tile([rows, cols], dt, tag=name)
                src = din[name].ap()
                bc = bass.AP(tensor=src.tensor, offset=0,
                             ap=[[0, rows]] + list(src.ap[1:]))
                nc.sync.dma_start(out=t[:], in_=bc)
                return t

            g1_b = bload("g1", C)
            b1ln_b = bload("b1ln", C)
            g2_b = bload("g2", C)
            b2ln_b = bload("b2ln", C)
            bq_b = bload("bq", C)
            bk_b = bload("bk", C)
            bv_b = bload("bv", C)
            bo_b = bload("bo", C)
            b1f_b = bload("b1f", FF)
            b2f_b = bload("b2f", C)
            iota_b = bload("iota_r", 128, dt=BF16)

            def wload(name, cols, shape=None):
                t = cp.tile(shape or [C, cols], BF16, tag=name)
                nc.sync.dma_start(out=t[:], in_=din[name][:])
                return t

            wq_s = wload("wq", C)
            wk_s = wload("wk", C)
            wv_s = wload("wv", C)
            wo_s = wload("wo", C)
            w1_s = wload("w1", FF)
            w2_s = cp.tile([C, 4 * C], BF16, tag="w2")
            for j in range(4):
                nc.sync.dma_start(out=w2_s[:, j * C:(j + 1) * C],
                                  in_=din["w2s"][j])
            ident_s = wload("identb", 128, shape=[128, 128])
            expand_s = cp.tile([32, 128], F32, tag="expand")
            nc.sync.dma_start(out=expand_s[:], in_=din["expand"][:])

            eps_t = cp.tile([128, 1], F32, tag="eps")
            nc.vector.memset(eps_t[:], EPS)
            ones_c = cp.tile([128, 1], BF16, tag="ones")
            nc.vector.memset(ones_c[:], 1.0)
            z_sb = pp.tile([32, 1], F32, tag="z_sb")
            nc.vector.memset(z_sb[:], 0.0)

            x_tiles = [pp.tile([128, C], F32, name=f"x{i}", tag=f"x{i}")
                       for i in range(NCH)]
            q_tiles = [pp.tile([128, C], BF16, name=f"q{i}", tag=f"q{i}")
                       for i in range(NCH)]
            a_tiles = [pp.tile([128, C], F32, name=f"a{i}", tag=f"a{i}")
                       for i in range(NCH)]

            def layernorm(x_t, out_t, g_b, b_b):
                st = wp.tile([128, 6], F32, tag="ln_st")
                mv = wp.tile([128, 2], F32, tag="ln_mv")
                nc.vector.bn_stats(out=st[:], in_=x_t[:])
                nc.vector.bn_aggr(out=mv[:], in_=st[:])
                std = wp.tile([128, 1], F32, tag="ln_std")
                nc.scalar.activation(out=std[:], in_=mv[:, 1:2], func=AF.Sqrt,
                                     bias=eps_t[:], scale=1.0)
                nc.vector.reciprocal(out=std[:], in_=std[:])
                tmp = wp.tile([128, C], F32, tag="ln_tmp")
                nc.vector.tensor_scalar(out=tmp[:], in0=x_t[:],
                                        scalar1=mv[:, 0:1], scalar2=std[:],
                                        op0=OP.subtract, op1=OP.mult)
                nc.vector.tensor_mul(out=tmp[:], in0=tmp[:], in1=g_b[:])
                nc.vector.tensor_add(out=out_t[:], in0=tmp[:], in1=b_b[:])

            # ================= Phase A =================
            for i in range(NCH):
                x_t = x_tiles[i]
                nc.sync.dma_start(out=x_t[:],
                                  in_=din["x_loc"][i * 128:(i + 1) * 128, :])
                xn = wp.tile([128, C], BF16, tag="xn")
                layernorm(x_t, xn, g1_b, b1ln_b)
                xnT_p = ps_t.tile([128, C], BF16, tag="pt")
                nc.tensor.transpose(out=xnT_p[:], in_=xn[:], identity=ident_s[:])
                xnT = wp.tile([128, C], BF16, tag="xnT")
                nc.scalar.activation(out=xnT[:], in_=xnT_p[:], func=AF.Copy)

                q_p = ps_qg.tile([128, C], F32, tag="pqg")
                nc.tensor.matmul(out=q_p[:], lhsT=xnT[:], rhs=wq_s[:],
                                 start=True, stop=True)
                nc.vector.tensor_add(out=q_tiles[i][:], in0=q_p[:], in1=bq_b[:])

                k_p = ps_agg.tile([128, C], F32, tag="pagg")
                nc.tensor.matmul(out=k_p[:], lhsT=xnT[:], rhs=wk_s[:],
                                 start=True, stop=True)
                kv_t = wp.tile([128, 2 * C], BF16, tag="kv_t")
                nc.vector.tensor_add(out=kv_t[:, 0:C], in0=k_p[:], in1=bk_b[:])

                v_p = ps_big.tile([128, C], F32, tag="pbig")
                nc.tensor.matmul(out=v_p[:], lhsT=xnT[:], rhs=wv_s[:],
                                 start=True, stop=True)
                nc.vector.tensor_add(out=kv_t[:, C:2 * C], in0=v_p[:], in1=bv_b[:])

                nc.sync.dma_start(
                    out=kv_bounce[i * 128:(i + 1) * 128, :], in_=kv_t[:])

            # ================= AllGather kv =================
            if KABL != "nocoll":
                nc.gpsimd.collective_compute(
                    "AllGather", OP.bypass,
                    replica_groups=[list(range(M))],
                    ins=[kv_bounce[:].opt()],
                    outs=[kv_full[:].opt()],
                )

            # ================= Phase B: edges =================
            for c in range(KLIMIT):
                idxa_t = wp.tile([128, cap_a // 16], I16, tag="idxa")
                nc.sync.dma_start(out=idxa_t[:], in_=din["idx_a"][c])
                idxb_t = wp.tile([128, cap_b // 16], I16, tag="idxb")
                nc.sync.dma_start(out=idxb_t[:], in_=din["idx_b"][c])
                dstc_t = wp.tile([128, tt], BF16, tag="dstc")
                nc.sync.dma_start(out=dstc_t[:], in_=din["dstc"][c])
                mask_t = wp.tile([128, tt], BF16, tag="maskt")
                nc.sync.dma_start(out=mask_t[:], in_=din["mask"][c])

                kvg = kvp.tile([128, tt * 2 * C], BF16, tag="kvg")
                kvg3 = kvg[:].rearrange("p (t c) -> p t c", t=tt)
                if KABL == "nogather":
                    for t in range(tt):
                        nc.sync.dma_start(
                            out=kvg3[:, t, :],
                            in_=kv_full[t * 128:(t + 1) * 128, :])
                else:
                    nc.gpsimd.dma_gather(
                        out_ap=kvg3[:, 0:ta, :], in_ap=kv_full[:],
                        idxs_ap=idxa_t[:], num_idxs=cap_a, num_idxs_reg=cap_a,
                        elem_size=2 * C, single_packet=cap_a <= 1024)
                    nc.gpsimd.dma_gather(
                        out_ap=kvg3[:, ta:tt, :], in_ap=kv_full[SPLIT:, :],
                        idxs_ap=idxb_t[:], num_idxs=cap_b, num_idxs_reg=cap_b,
                        elem_size=2 * C, single_packet=cap_b <= 1024)

                aggT_p = ps_agg.tile([128, 128], F32, tag="pagg")
                z_p = ps_z.tile([32, 1], F32, tag="z_p")
                q_c = q_tiles[c]
                inner_tt = 0 if KABL == "noinner" else tt
                groups = []
                t0 = 0
                while t0 < inner_tt:
                    groups.append((t0, min(GB, inner_tt - t0)))
                    t0 += GB
                for gi, (t0, bs) in enumerate(groups):
                    w = bs * 128
                    # one-hot for bs tiles in one DVE op: [128, bs, 128] bf16
                    oh4 = e4.tile([128, GB * 128], BF16, tag="oh4")
                    dap = dstc_t[:, t0:t0 + bs]
                    in0 = bass.AP(tensor=dap.tensor, offset=dap.offset,
                                  ap=list(dap.ap) + [[0, 128]])
                    iap = iota_b[:]
                    in1 = bass.AP(tensor=iap.tensor, offset=iap.offset,
                                  ap=[list(iap.ap[0]), [0, bs], list(iap.ap[1])])
                    oh4v = oh4[:, 0:w]
                    nc.vector.tensor_tensor(
                        out=oh4v.rearrange("p (b i) -> p b i", b=bs),
                        in0=in0, in1=in1, op=OP.is_equal)

                    # bs PE transposes into one PSUM bank, ACT copies out
                    ohT_p = ps_t.tile([128, GB * 128], BF16, tag="pt")
                    for j in range(bs):
                        nc.tensor.transpose(
                            out=ohT_p[:, j * 128:(j + 1) * 128],
                            in_=oh4[:, j * 128:(j + 1) * 128],
                            identity=ident_s[:])
                    ohT4 = e4.tile([128, GB * 128], BF16, tag="ohT4")
                    nc.scalar.activation(out=ohT4[:, 0:w], in_=ohT_p[:, 0:w],
                                         func=AF.Copy)

                    # bs q-expansion matmuls into one PSUM bank
                    qg_p = ps_qg.tile([128, GB * 128], F32, tag="pqg")
                    for j in range(bs):
                        nc.tensor.matmul(out=qg_p[:, j * 128:(j + 1) * 128],
                                         lhsT=ohT4[:, j * 128:(j + 1) * 128],
                                         rhs=q_c[:], start=True, stop=True)
                    qs4 = e4.tile([128, GB * 128], BF16, tag="qs4")
                    nc.scalar.activation(out=qs4[:, 0:w], in_=qg_p[:, 0:w],
                                         func=AF.Copy)

                    prod4 = e4.tile([128, GB * 128], BF16, tag="prod4")
                    s84 = e4.tile([128, GB * H], F32, tag="s84")
                    nc.vector.tensor_mul(
                        out=prod4[:, 0:w], in0=qs4[:, 0:w],
                        in1=kvg3[:, t0:t0 + bs, 0:C])
                    nc.vector.reduce_sum(
                        out=s84[:, 0:bs * H].rearrange("p (b h) -> p b h", b=bs),
                        in_=prod4[:, 0:w].rearrange("p (b h d) -> p b h d",
                                                    b=bs, h=H),
                        axis=mybir.AxisListType.X)
                    w84 = e4.tile([128, GB * H], BF16, tag="w84")
                    nc.scalar.activation(out=w84[:, 0:bs * H],
                                         in_=s84[:, 0:bs * H], func=AF.Exp,
                                         scale=1.0 / math.sqrt(D))
                    # mask: per (p, tile) broadcast over heads
                    map_ = mask_t[:, t0:t0 + bs]
                    mbc = bass.AP(tensor=map_.tensor, offset=map_.offset,
                                  ap=list(map_.ap) + [[0, H]])
                    w84m = e4.tile([128, GB * H], BF16, tag="w84m")
                    nc.vector.tensor_tensor(
                        out=w84m[:, 0:bs * H].rearrange("p (b h) -> p b h", b=bs),
                        in0=w84[:, 0:bs * H].rearrange("p (b h) -> p b h", b=bs),
                        in1=mbc, op=OP.mult)
                    if KABL != "noz":
                        # Z: per-(b,h) sums of w over edges via N=1 matmul
                        nc.tensor.matmul(out=z_p[0:bs * H, :],
                                         lhsT=w84m[:, 0:bs * H],
                                         rhs=ones_c[:],
                                         start=(gi == 0),
                                         stop=(gi == len(groups) - 1))

                    wt4 = e4.tile([128, GB * 128], BF16, tag="wt4")
                    w8ap = w84m[:, 0:bs * H]
                    w8b = bass.AP(
                        tensor=w8ap.tensor, offset=w8ap.offset,
                        ap=[list(w8ap.ap[0]), [H, bs], [1, H], [0, D]])
                    nc.vector.tensor_tensor(
                        out=wt4[:, 0:w].rearrange("p (b h d) -> p b h d",
                                                  b=bs, h=H),
                        in0=kvg3[:, t0:t0 + bs, C:2 * C].rearrange(
                            "p b (h d) -> p b h d", h=H),
                        in1=w8b, op=OP.mult)

                    for j in range(bs):
                        t = t0 + j
                        nc.tensor.matmul(out=aggT_p[:],
                                         lhsT=wt4[:, j * 128:(j + 1) * 128],
                                         rhs=oh4[:, j * 128:(j + 1) * 128],
                                         start=(t == 0), stop=(t == inner_tt - 1))

                if KABL == "noinner":
                    nc.vector.memset(a_tiles[c][:], 0.0)
                else:
                    nc.scalar.activation(out=a_tiles[c][:], in_=aggT_p[:],
                                         func=AF.Copy)
                    if KABL != "noz":
                        nc.vector.tensor_add(out=z_sb[:], in0=z_sb[:],
                                             in1=z_p[:])

            # ================= Z reduce =================
            nc.sync.dma_start(out=z_bounce[:], in_=z_sb[:])
            if KABL != "nocoll":
                nc.gpsimd.collective_compute(
                    "AllReduce", OP.add,
                    replica_groups=[list(range(M))],
                    ins=[z_bounce[:].opt()],
                    outs=[z_red[:].opt()],
                )
            z_red_s = wp.tile([32, 1], F32, tag="z_red_s")
            nc.sync.dma_start(out=z_red_s[:], in_=z_red[:])
            zb_p = ps_z.tile([128, 1], F32, tag="z_p")
            nc.tensor.matmul(out=zb_p[:], lhsT=expand_s[:], rhs=z_red_s[:],
                             start=True, stop=True)
            zrecip = cp.tile([128, 1], F32, tag="zrecip")
            nc.vector.tensor_copy(out=zrecip[:], in_=zb_p[:])
            nc.vector.reciprocal(out=zrecip[:], in_=zrecip[:])

            # ================= Phase C =================
            for c in range(KLIMIT):
                sagg = wp.tile([128, 128], BF16, tag="sagg")
                nc.vector.tensor_scalar_mul(out=sagg[:], in0=a_tiles[c][:],
                                            scalar1=zrecip[:])
                o_p = ps_qg.tile([128, C], F32, tag="pqg")
                nc.tensor.matmul(out=o_p[:], lhsT=sagg[:], rhs=wo_s[:],
                                 start=True, stop=True)
                x1 = wp.tile([128, C], F32, tag="x1")
                nc.vector.tensor_add(out=x1[:], in0=o_p[:], in1=bo_b[:])
                nc.vector.tensor_add(out=x1[:], in0=x1[:], in1=x_tiles[c][:])

                xn2 = wp.tile([128, C], BF16, tag="xn2")
                layernorm(x1, xn2, g2_b, b2ln_b)
                xn2T_p = ps_t.tile([128, C], BF16, tag="pt")
                nc.tensor.transpose(out=xn2T_p[:], in_=xn2[:],
                                    identity=ident_s[:])
                xn2T = wp.tile([128, C], BF16, tag="xn2T")
                nc.scalar.activation(out=xn2T[:], in_=xn2T_p[:], func=AF.Copy)

                h_p = ps_big.tile([128, FF], F32, tag="pbig")
                nc.tensor.matmul(out=h_p[:], lhsT=xn2T[:], rhs=w1_s[:],
                                 start=True, stop=True)
                hb = wp.tile([128, FF], BF16, tag="hb")
                nc.vector.tensor_add(out=hb[:], in0=h_p[:], in1=b1f_b[:])
                hbr = wp.tile([128, FF], BF16, tag="hbr")
                nc.scalar.activation(out=hbr[:], in_=hb[:], func=AF.Relu)

                hT = wp.tile([128, FF], BF16, tag="hT")
                for j in range(4):
                    hT_p = ps_t.tile([128, C], BF16, tag="pt")
                    nc.tensor.transpose(out=hT_p[:],
                                        in_=hbr[:, j * C:(j + 1) * C],
                                        identity=ident_s[:])
                    nc.scalar.activation(out=hT[:, j * C:(j + 1) * C],
                                         in_=hT_p[:], func=AF.Copy)
                f_p = ps_agg.tile([128, C], F32, tag="pagg")
                for j in range(4):
                    nc.tensor.matmul(out=f_p[:], lhsT=hT[:, j * C:(j + 1) * C],
                                     rhs=w2_s[:, j * C:(j + 1) * C],
                                     start=(j == 0), stop=(j == 3))
                fin = wp.tile([128, C], F32, tag="fin")
                nc.vector.tensor_add(out=fin[:], in0=f_p[:], in1=b2f_b[:])
                nc.vector.tensor_add(out=fin[:], in0=fin[:], in1=x1[:])
                nc.sync.dma_start(out=out_d[c * 128:(c + 1) * 128, :],
                                  in_=fin[:])

    nc.compile()
    return nc


_CACHE = {}


def _make_in_maps(inputs, x_loc, idx_a, idx_b, dstc, mask):
    f32 = lambda a: np.ascontiguousarray(np.asarray(a, dtype=np.float32))
    bf16 = lambda a: np.ascontiguousarray(
        np.asarray(a, dtype=np.float32).astype(BF))
    expand = np.zeros((32, 128), dtype=np.float32)
    for b in range(4):
        for h in range(H):
            expand[b * 8 + h, h * D:(h + 1) * D] = 1.0

    shared = dict(
        wq=bf16(inputs["Wq"]), wk=bf16(inputs["Wk"]), wv=bf16(inputs["Wv"]),
        wo=bf16(inputs["Wo"]), w1=bf16(inputs["W1"]),
        w2s=bf16(np.asarray(inputs["W2"], dtype=np.float32).reshape(4, C, C)),
        g1=f32(inputs["ln1_g"]).reshape(1, C),
        b1ln=f32(inputs["ln1_b"]).reshape(1, C),
        g2=f32(inputs["ln2_g"]).reshape(1, C),
        b2ln=f32(inputs["ln2_b"]).reshape(1, C),
        bq=f32(inputs["bq"]).reshape(1, C), bk=f32(inputs["bk"]).reshape(1, C),
        bv=f32(inputs["bv"]).reshape(1, C), bo=f32(inputs["bo"]).reshape(1, C),
        b1f=f32(inputs["b1"]).reshape(1, FF),
        b2f=f32(inputs["b2"]).reshape(1, C),
        identb=np.eye(128, dtype=np.float32).astype(BF),
        iota_r=np.arange(128, dtype=np.float32).astype(BF).reshape(1, 128),
        expand=expand,
    )
    return [
        dict(shared, x_loc=x_loc[m], idx_a=idx_a[m], idx_b=idx_b[m],
             dstc=dstc[m], mask=mask[m])
        for m in range(M)
    ]


def kernel(**inputs):
    x = np.asarray(inputs["x"], dtype=np.float32)
    edge_index = np.asarray(inputs["edge_index"])

    x_loc, idx_a, idx_b, dstc, mask, cap_a, cap_b = _preprocess(x, edge_index)

    key = (cap_a, cap_b)
    if key not in _CACHE:
        _CACHE[key] = _build(cap_a, cap_b)
    nc = _CACHE[key]

    in_maps = _make_in_maps(inputs, x_loc, idx_a, idx_b, dstc, mask)
    res = bass_utils.run_bass_kernel_spmd(nc, in_maps, core_ids=list(range(M)))
    out = np.concatenate([res.results[m]["out"][:NPC] for m in range(M)], axis=0)
    return out.astype(np.float32)
